# revision 1
# baseline (speedup 1.0000x reference)
"""CDMamba Trainium2 kernel.

Sharding: 8 cores = 4 batches x 2 halves of d_inner (512 channels each).
Each core computes the full x-path (in_proj x-part, conv, silu, x_proj) for
all 1024 channels (duplicated across the pair, so the d_inner contraction in
x_proj needs no collective); the z-path, dt, selective scan, gating and
out_proj run on the local 512 channels only.  out_proj partials (contraction
over d_inner) are summed on the host.  The per-core channel order is
permuted on the host so the local half is always channels 0..511 — the
program is SPMD-identical, only the data differs.

On-chip layout is d-major: [d partitions, time free].  The selective scan
uses the DVE tensor_tensor_scan instruction (state = a_t*state + u_t per
partition row), one scan per (d-tile, state index n).  The n-contraction
y = sum_n C_n * h_n accumulates in PSUM via identity matmuls on the tensor
engine.  The reverse branch runs on a host-time-reversed copy of the input
and its output is un-reversed on chip in phase C.
"""

import sys

import numpy as np

for _p in ("/opt/trn_rl_repo",):
    if _p not in sys.path:
        sys.path.insert(0, _p)

import concourse.bass as bass
import concourse.bacc as bacc
import concourse.tile as tile
from concourse import library_config
from concourse import mybir
from concourse.bass_utils import run_bass_kernel_spmd

F16 = mybir.dt.float16
F32 = mybir.dt.float32
F32R = mybir.dt.float32r
AF = mybir.ActivationFunctionType
OP = mybir.AluOpType

D_MODEL = 512
D_STATE = 16
D_CONV = 4
D_INNER = 1024
DT_RANK = 32
NB = 4
NCORE = 8
DLOC = 512
L_FULL = 4096

BRANCHES = ("f", "r", "g")


def build(L, TA=512, TB=1024, scan_pool_n=0):
    """Build the SPMD Bass program for sequence length L."""
    TA = min(TA, L)
    TB = min(TB, L)
    assert L % TA == 0 and L % TB == 0
    nlt_a = L // TA
    nlt_b = L // TB
    nqb = TB // 512

    nc = bacc.Bacc()

    # ---- I/O ----
    inp = {t: nc.declare_dram_parameter(f"inp_{t}", [D_MODEL, L], F16, isOutput=False) for t in BRANCHES}
    Wx = nc.declare_dram_parameter("Wx", [D_MODEL, D_INNER], F16, isOutput=False)
    Wz = nc.declare_dram_parameter("Wz", [D_MODEL, DLOC], F16, isOutput=False)
    Gx = nc.declare_dram_parameter("Gx", [D_MODEL, D_INNER], F16, isOutput=False)
    Gz = nc.declare_dram_parameter("Gz", [D_MODEL, DLOC], F16, isOutput=False)
    cw = {t: nc.declare_dram_parameter(f"cw_{t}", [128, 32], F32, isOutput=False) for t in BRANCHES}
    cb = {t: nc.declare_dram_parameter(f"cb_{t}", [128, 8], F32, isOutput=False) for t in BRANCHES}
    xp = {t: nc.declare_dram_parameter(f"xp_{t}", [D_INNER, 64], F16, isOutput=False) for t in BRANCHES}
    dtw = {t: nc.declare_dram_parameter(f"dtw_{t}", [DT_RANK, DLOC], F32, isOutput=False) for t in BRANCHES}
    dtb = {t: nc.declare_dram_parameter(f"dtb_{t}", [128, 4], F32, isOutput=False) for t in BRANCHES}
    An = {t: nc.declare_dram_parameter(f"An_{t}", [128, 64], F32, isOutput=False) for t in BRANCHES}
    Dv = {t: nc.declare_dram_parameter(f"Dv_{t}", [128, 4], F32, isOutput=False) for t in BRANCHES}
    opw = nc.declare_dram_parameter("opw", [DLOC, D_MODEL], F16, isOutput=False)
    ident = nc.declare_dram_parameter("ident", [128, 128], F16, isOutput=False)
    outp = nc.declare_dram_parameter("outp", [D_MODEL, L], F32, isOutput=True)

    # ---- DRAM scratch ----
    xs_d = {t: nc.dram_tensor(f"xs_{t}", [DLOC, L], F16) for t in BRANCHES}
    szl_d = {t: nc.dram_tensor(f"szl_{t}", [DLOC, L], F16) for t in BRANCHES}
    y_d = {t: nc.dram_tensor(f"y_{t}", [DLOC, L], F16) for t in BRANCHES}
    ddt_d = {t: nc.dram_tensor(f"ddt_{t}", [DT_RANK, L], F32) for t in BRANCHES}
    bc_d = {t: nc.dram_tensor(f"bc_{t}", [32, L], F16) for t in BRANCHES}

    with tile.TileContext(nc) as tc:
        with tc.tile_pool(name="wpool", bufs=1) as wp:
            # ---- persistent weights in SBUF ----
            def load_w(pool, name, dram, parts, width, dtype=F16):
                tl = []
                for k in range(parts):
                    t_ = pool.tile([128, width], dtype, tag=f"{name}{k}", name=f"{name}{k}")
                    nc.sync.dma_start(t_[:], dram[k * 128:(k + 1) * 128, :])
                    tl.append(t_)
                return tl

            xp_sb, dtw_sb, cw_sb, cb_sb, dtb_sb, An_sb, Dv_sb = {}, {}, {}, {}, {}, {}, {}
            for t in BRANCHES:
                xp_sb[t] = wp.tile([128, 8 * 64], F16, tag=f"xp_{t}", name=f"xp_{t}")
                for k in range(8):
                    nc.sync.dma_start(xp_sb[t][:, k * 64:(k + 1) * 64],
                                      xp[t][k * 128:(k + 1) * 128, :])
                dtw_sb[t] = wp.tile([DT_RANK, DLOC], F32, tag=f"dtw_{t}", name=f"dtw_{t}")
                nc.sync.dma_start(dtw_sb[t][:], dtw[t][:])
                for nm, src, shape in (("cw", cw, [128, 32]), ("cb", cb, [128, 8]),
                                       ("dtb", dtb, [128, 4]), ("An", An, [128, 64]),
                                       ("Dv", Dv, [128, 4])):
                    t_ = wp.tile(shape, F32, tag=f"{nm}_{t}")
                    nc.sync.dma_start(t_[:], src[t][:])
                    {"cw": cw_sb, "cb": cb_sb, "dtb": dtb_sb, "An": An_sb, "Dv": Dv_sb}[nm][t] = t_
            op_sb = load_w(wp, "op", opw, 4, D_MODEL)
            id_sb = wp.tile([128, 128], F16, tag="ident", name="ident")
            nc.sync.dma_start(id_sb[:], ident[:])
            tc.strict_bb_all_engine_barrier()

            # ================= PHASE A: projections + conv + silu + dbl ====
            with tc.tile_pool(name="wa", bufs=1) as wa, \
                 tc.tile_pool(name="pa", bufs=2) as pa, \
                 tc.tile_pool(name="pa_ps", bufs=3, space="PSUM") as pa_ps, \
                 tc.tile_pool(name="pa_dbl", bufs=2, space="PSUM") as pa_dbl:
                wx_sb = {"Wx": load_w(wa, "Wx", Wx, 4, D_INNER),
                         "Gx": load_w(wa, "Gx", Gx, 4, D_INNER)}
                wz_sb = {"Wz": load_w(wa, "Wz", Wz, 4, DLOC),
                         "Gz": load_w(wa, "Gz", Gz, 4, DLOC)}
                in_x_sb = {"f": wx_sb["Wx"], "r": wx_sb["Wx"], "g": wx_sb["Gx"]}
                in_z_sb = {"f": wz_sb["Wz"], "r": wz_sb["Wz"], "g": wz_sb["Gz"]}
                tc.strict_bb_all_engine_barrier()
                for t in BRANCHES:
                    xz_prev = [None] * 8
                    for lt in range(nlt_a):
                        t0 = lt * TA
                        rhs = []
                        for k in range(4):
                            r_ = pa.tile([128, TA], F16, tag=f"rhs{k}", name=f"rhs{k}")
                            nc.sync.dma_start(r_[:], inp[t][k * 128:(k + 1) * 128, t0:t0 + TA])
                            rhs.append(r_)
                        dbl_ps = pa_dbl.tile([64, TA], F32, tag="dbl", name="dbl")
                        for d8 in range(8):
                            ps = pa_ps.tile([128, TA], F32, tag="xzps", name="xzps")
                            for k in range(4):
                                nc.tensor.matmul(ps[:], in_x_sb[t][k][:, d8 * 128:(d8 + 1) * 128],
                                                 rhs[k][:], start=(k == 0), stop=(k == 3))
                            xz = pa.tile([128, TA + 3], F16, tag=f"xz{d8}", name=f"xz{d8}")
                            if lt == 0:
                                nc.gpsimd.memset(xz[:, 0:3], 0.0)
                            else:
                                nc.vector.tensor_copy(xz[:, 0:3], xz_prev[d8][:, TA:TA + 3])
                            nc.scalar.copy(xz[:, 3:TA + 3], ps[:])
                            xz_prev[d8] = xz
                            tps = []
                            s01 = s23 = None
                            for k in range(4):
                                tp = pa.tile([128, TA], F16, tag=f"cvt{k % 2}", name=f"cvt{k % 2}")
                                nc.vector.tensor_scalar_mul(tp[:], xz[:, k:k + TA],
                                                            cw_sb[t][:, d8 * 4 + k:d8 * 4 + k + 1])
                                tps.append(tp)
                                if k == 1:
                                    s01 = pa.tile([128, TA], F16, tag="cva", name="cva")
                                    nc.vector.tensor_add(s01[:], tps[0][:], tps[1][:])
                                if k == 3:
                                    s23 = pa.tile([128, TA], F16, tag="cvb", name="cvb")
                                    nc.vector.tensor_add(s23[:], tps[2][:], tps[3][:])
                            cv = pa.tile([128, TA], F16, tag="cvc", name="cvc")
                            nc.vector.scalar_tensor_tensor(cv[:], s01[:],
                                                           cb_sb[t][:, d8:d8 + 1], s23[:],
                                                           op0=OP.add, op1=OP.add)
                            sgm = pa.tile([128, TA], F16, tag="sgm", name="sgm")
                            nc.scalar.activation(sgm[:], cv[:], AF.Sigmoid)
                            xs = pa.tile([128, TA], F16, tag="xs", name="xs")
                            nc.vector.tensor_mul(xs[:], cv[:], sgm[:])
                            nc.tensor.matmul(dbl_ps[:], xp_sb[t][:, d8 * 64:(d8 + 1) * 64],
                                             xs[:], start=(d8 == 0), stop=(d8 == 7))
                            if d8 < 4:  # local half (host permutes channels)
                                nc.sync.dma_start(xs_d[t][d8 * 128:(d8 + 1) * 128, t0:t0 + TA],
                                                  xs[:])
                        for zt in range(4):
                            zps = pa_ps.tile([128, TA], F32, tag="xzps", name="xzps")
                            for k in range(4):
                                nc.tensor.matmul(zps[:], in_z_sb[t][k][:, zt * 128:(zt + 1) * 128],
                                                 rhs[k][:], start=(k == 0), stop=(k == 3))
                            zv = pa.tile([128, TA], F16, tag="zv", name="zv")
                            nc.scalar.copy(zv[:], zps[:])
                            zs = pa.tile([128, TA], F16, tag="zs", name="zs")
                            nc.scalar.activation(zs[:], zps[:], AF.Sigmoid)
                            sz = pa.tile([128, TA], F16, tag="sz", name="sz")
                            nc.vector.tensor_mul(sz[:], zv[:], zs[:])
                            nc.sync.dma_start(szl_d[t][zt * 128:(zt + 1) * 128, t0:t0 + TA], sz[:])
                        ddt_t = pa.tile([DT_RANK, TA], F32, tag="ddt", name="ddt")
                        nc.scalar.copy(ddt_t[:], dbl_ps[0:DT_RANK, :])
                        nc.sync.dma_start(ddt_d[t][:, t0:t0 + TA], ddt_t[:])
                        bc_t = pa.tile([32, TA], F16, tag="bct", name="bct")
                        nc.scalar.copy(bc_t[:], dbl_ps[DT_RANK:64, :])
                        nc.sync.dma_start(bc_d[t][:, t0:t0 + TA], bc_t[:])

            # ================= PHASE B: selective scan =====================
            with tc.tile_pool(name="pb", bufs=2) as pb, \
                 tc.tile_pool(name="pbc", bufs=1) as pbc, \
                 tc.tile_pool(name="pb_ps", bufs=2, space="PSUM") as pb_ps, \
                 tc.tile_pool(name="pb_yps", bufs=2, space="PSUM") as pb_yps:
                for t in BRANCHES:
                    carries = [None] * 4
                    for lt in range(nlt_b):
                        t0 = lt * TB
                        ddt_sb = pb.tile([DT_RANK, TB], F32, tag="ddt_sb", name="ddt_sb")
                        nc.sync.dma_start(ddt_sb[:], ddt_d[t][:, t0:t0 + TB])
                        dt_t, dtx_t, xs_t = [], [], []
                        for dti in range(4):
                            xst = pbc.tile([128, TB], F16, tag=f"xs{dti}", name=f"xs{dti}")
                            nc.sync.dma_start(xst[:], xs_d[t][dti * 128:(dti + 1) * 128, t0:t0 + TB])
                            xs_t.append(xst)
                            dtt = pbc.tile([128, TB], F16, tag=f"dt{dti}", name=f"dt{dti}")
                            for q in range(nqb):
                                dps = pb_ps.tile([128, 512], F32, tag="dtps", name="dtps")
                                nc.tensor.matmul(dps[:],
                                                 dtw_sb[t][:, dti * 128:(dti + 1) * 128],
                                                 ddt_sb[:, q * 512:(q + 1) * 512],
                                                 start=True, stop=True)
                                # softplus(x) = ln(exp(x) + 1); x in [-8, 2] here
                                spe = pb.tile([128, 512], F32, tag="spe", name="spe", bufs=1)
                                nc.scalar.activation(spe[:], dps[:], AF.Exp,
                                                     bias=dtb_sb[t][:, dti:dti + 1])
                                nc.scalar.activation(dtt[:, q * 512:(q + 1) * 512], spe[:],
                                                     AF.Ln, bias=1.0)
                            dt_t.append(dtt)
                            dxt = pbc.tile([128, TB], F16, tag=f"dtx{dti}", name=f"dtx{dti}")
                            nc.vector.tensor_mul(dxt[:], dtt[:], xst[:])
                            dtx_t.append(dxt)
                        bn_t, cn_t = [], []
                        for n in range(D_STATE):
                            bn = pbc.tile([128, TB], F16, tag=f"Bn{n}", name=f"Bn{n}")
                            cn = pbc.tile([128, TB], F16, tag=f"Cn{n}", name=f"Cn{n}")
                            nc.sync.dma_start(
                                bn[:], bc_d[t][n:n + 1, t0:t0 + TB].partition_broadcast(128))
                            nc.sync.dma_start(
                                cn[:], bc_d[t][16 + n:17 + n, t0:t0 + TB].partition_broadcast(128))
                            bn_t.append(bn)
                            cn_t.append(cn)
                        for dti in range(4):
                            yps = pb_yps.tile([128, TB], F32, tag="yps", name="yps")
                            cnew = pb.tile([128, D_STATE], F32, tag=f"carry{dti}",
                                           name=f"carry{dti}")
                            for n in range(D_STATE):
                                a_t = pb.tile([128, TB], F32, tag="a", name="a")
                                nc.scalar.activation(a_t[:], dt_t[dti][:], AF.Exp,
                                                     scale=An_sb[t][:, dti * 16 + n:dti * 16 + n + 1])
                                u_t = pb.tile([128, TB], F16, tag="u", name="u")
                                nc.vector.tensor_mul(u_t[:], dtx_t[dti][:], bn_t[n][:])
                                h_t = pb.tile([128, TB], F16, tag="h", name="h")
                                init = 0.0 if lt == 0 else carries[dti][:, n:n + 1]
                                eng = nc.gpsimd if n >= D_STATE - scan_pool_n else nc.vector
                                eng.tensor_tensor_scan(h_t[:], a_t[:], u_t[:], init,
                                                       op0=OP.mult, op1=OP.add)
                                nc.gpsimd.tensor_copy(cnew[:, n:n + 1], h_t[:, TB - 1:TB])
                                tmp = pb.tile([128, TB], F16, tag="tmp", name="tmp")
                                nc.vector.tensor_mul(tmp[:], h_t[:], cn_t[n][:])
                                for q in range(nqb):
                                    nc.tensor.matmul(yps[:, q * 512:(q + 1) * 512], id_sb[:],
                                                     tmp[:, q * 512:(q + 1) * 512],
                                                     start=(n == 0), stop=(n == D_STATE - 1))
                            carries[dti] = cnew
                            yD = pb.tile([128, TB], F16, tag="yD", name="yD")
                            nc.vector.scalar_tensor_tensor(yD[:], xs_t[dti][:],
                                                           Dv_sb[t][:, dti:dti + 1], yps[:],
                                                           op0=OP.mult, op1=OP.add)
                            szt = pb.tile([128, TB], F16, tag="szt", name="szt")
                            nc.sync.dma_start(szt[:],
                                              szl_d[t][dti * 128:(dti + 1) * 128, t0:t0 + TB])
                            yo = pb.tile([128, TB], F16, tag="yo", name="yo")
                            nc.vector.tensor_mul(yo[:], yD[:], szt[:])
                            nc.sync.dma_start(y_d[t][dti * 128:(dti + 1) * 128, t0:t0 + TB], yo[:])

            # ================= PHASE C: combine + out_proj ==================
            with tc.tile_pool(name="pc", bufs=2) as pc, \
                 tc.tile_pool(name="pc_ps", bufs=3, space="PSUM") as pc_ps:
                for lt in range(nlt_b):
                    t0 = lt * TB
                    Y_t = []
                    for dti in range(4):
                        yf = pc.tile([128, TB], F16, tag="yf", name="yf")
                        nc.sync.dma_start(yf[:], y_d["f"][dti * 128:(dti + 1) * 128, t0:t0 + TB])
                        yg = pc.tile([128, TB], F16, tag="yg", name="yg")
                        nc.sync.dma_start(yg[:], y_d["g"][dti * 128:(dti + 1) * 128, t0:t0 + TB])
                        yrr = pc.tile([128, TB], F16, tag="yrr", name="yrr")
                        rt0 = L - t0 - TB
                        nc.sync.dma_start(yrr[:], y_d["r"][dti * 128:(dti + 1) * 128, rt0:rt0 + TB])
                        yr = pc.tile([128, TB], F16, tag="yr", name="yr")
                        src = yrr[:]
                        rev = bass.AP(tensor=src.tensor, offset=src.offset + (TB - 1),
                                      ap=[list(src.ap[0]), [-1, TB]])
                        nc.vector.tensor_copy(yr[:], rev)
                        sgs = pc.tile([128, TB], F16, tag="sgs", name="sgs")
                        nc.scalar.activation(sgs[:], yg[:], AF.Sigmoid)
                        sg = pc.tile([128, TB], F16, tag="sg", name="sg")
                        nc.vector.tensor_mul(sg[:], yg[:], sgs[:])
                        fr = pc.tile([128, TB], F16, tag="fr", name="fr")
                        nc.vector.tensor_add(fr[:], yf[:], yr[:])
                        Y = pc.tile([128, TB], F16, tag=f"Y{dti}", name=f"Y{dti}")
                        nc.vector.tensor_mul(Y[:], fr[:], sg[:])
                        Y_t.append(Y)
                    for mt in range(4):
                        for q in range(nqb):
                            ops = pc_ps.tile([128, 512], F32, tag="ops", name="ops")
                            for dti in range(4):
                                nc.tensor.matmul(ops[:], op_sb[dti][:, mt * 128:(mt + 1) * 128],
                                                 Y_t[dti][:, q * 512:(q + 1) * 512],
                                                 start=(dti == 0), stop=(dti == 3))
                            ot = pc.tile([128, 512], F32, tag="ot", name="ot")
                            nc.scalar.copy(ot[:], ops[:])
                            nc.sync.dma_start(
                                outp[mt * 128:(mt + 1) * 128, t0 + q * 512:t0 + (q + 1) * 512],
                                ot[:])

    nc.finalize()
    return nc


def prep_core_inputs(inputs, c, L):
    """Build the input dict for core c (b = c//2, dh = c%2).

    Channels of d_inner are permuted per core so the local half is always
    first: perm = [dh*512 .. dh*512+511, other half].
    """
    b, dh = divmod(c, 2)
    f16 = np.float16
    f32 = np.float32
    perm = np.concatenate([np.arange(dh * 512, dh * 512 + 512),
                           np.arange((1 - dh) * 512, (1 - dh) * 512 + 512)])
    loc = perm[:512]

    hid = np.asarray(inputs["hidden_states"][b], dtype=f32)[:L]
    ano = np.asarray(inputs["another_hidden_states"][b], dtype=f32)[:L]
    d = {
        "inp_f": np.ascontiguousarray(hid.T).astype(f16),
        "inp_r": np.ascontiguousarray(hid[::-1].T).astype(f16),
        "inp_g": np.ascontiguousarray(ano.T).astype(f16),
        "Wx": np.ascontiguousarray(inputs["in_proj_w"][:D_INNER][perm].T).astype(f16),
        "Wz": np.ascontiguousarray(inputs["in_proj_w"][D_INNER:][loc].T).astype(f16),
        "Gx": np.ascontiguousarray(inputs["in_proj_g_w"][:D_INNER][perm].T).astype(f16),
        "Gz": np.ascontiguousarray(inputs["in_proj_g_w"][D_INNER:][loc].T).astype(f16),
        "opw": np.ascontiguousarray(inputs["out_proj_w"][:, loc].T).astype(f16),
        "ident": np.eye(128, dtype=f16),
    }
    for t, cwn, cbn, xpn, dtwn, dtbn, alogn, dn in (
            ("f", "convw_f", "convb_f", "xproj_f", "dtw_f", "dtb_f", "Alog_f", "D_f"),
            ("r", "convw_r", "convb_r", "xproj_r", "dtw_r", "dtb_r", "Alog_r", "D_r"),
            ("g", "convw_g", "convb_g", "xproj_g", "dtw_g", "dtb_g", "Alog_g", "D_g")):
        cwp = np.asarray(inputs[cwn], f32)[perm]          # (1024, 4)
        d[f"cw_{t}"] = np.ascontiguousarray(
            cwp.reshape(8, 128, 4).transpose(1, 0, 2).reshape(128, 32)).astype(f32)
        cbp = np.asarray(inputs[cbn], f32)[perm]          # (1024,)
        d[f"cb_{t}"] = np.ascontiguousarray(cbp.reshape(8, 128).T).astype(f32)
        d[f"xp_{t}"] = np.ascontiguousarray(np.asarray(inputs[xpn], f32).T[perm]).astype(f16)
        d[f"dtw_{t}"] = np.ascontiguousarray(np.asarray(inputs[dtwn], f32)[loc].T).astype(f32)
        dtbp = np.asarray(inputs[dtbn], f32)[loc]
        d[f"dtb_{t}"] = np.ascontiguousarray(dtbp.reshape(4, 128).T).astype(f32)
        Afull = -np.exp(np.asarray(inputs[alogn], f32))[loc]   # (512, 16)
        d[f"An_{t}"] = np.ascontiguousarray(
            Afull.reshape(4, 128, 16).transpose(1, 0, 2).reshape(128, 64)).astype(f32)
        Dp = np.asarray(inputs[dn], f32)[loc]
        d[f"Dv_{t}"] = np.ascontiguousarray(Dp.reshape(4, 128).T).astype(f32)
    return d


_NC_CACHE = {}
TRACE = False
LAST_RESULT = None


def kernel(**inputs):
    global LAST_RESULT
    L = inputs["hidden_states"].shape[1]
    key = L
    if key not in _NC_CACHE:
        _NC_CACHE[key] = build(L)
    nc = _NC_CACHE[key]
    in_maps = [prep_core_inputs(inputs, c, L) for c in range(NCORE)]
    res = run_bass_kernel_spmd(nc, in_maps, core_ids=list(range(NCORE)),
                               trace=TRACE)
    LAST_RESULT = res
    outs = []
    for b in range(NB):
        p = res.results[2 * b]["outp"].astype(np.float32) + \
            res.results[2 * b + 1]["outp"].astype(np.float32)
        outs.append(p.T)
    return np.stack(outs).astype(np.float32)


if __name__ == "__main__":
    nc = build(512)
    print("built ok")

